# revision 1
# baseline (speedup 1.0000x reference)
"""Multi-head attention TRN2 kernel (8 NeuronCores).

Problem: B=4, S=2048, D_IN=768, H=12, D_HEAD=64.
  q/k/v = einsum('hkd,bsk->bhsd', w{q,k,v}, x)
  out   = einsum('ij,bsj->bsi', wc, softmax(q k^T / 8) v  concat-heads)

Sharding: 8 cores = (batch b in 0..3) x (head-half in 0..1), 6 heads per core.
Each core computes, per head: Q^T,K^T [64,2048] fp16 projections, a
max-finding scores pass in [l,n] layout, a second scores pass in [n,l]
layout with the per-row max folded in via an augmented contraction row,
exp on the scalar engine (PSUM fp32 -> SBUF fp16), and attn@V with an
appended ones-column producing unnormalized z^T plus the softmax
normalizer Z.  The host divides by Z, concatenates heads and applies the
(tiny) output projection in fp32 BLAS.

All matmuls run in fp16 (fp32 PSUM accumulation).  Verified numerics of
this exact scheme vs the fp32 reference: rel_l2 = 1.2e-3.
"""

import numpy as np

B, S, D_IN, H, D_HEAD = 4, 2048, 768, 12, 64
HL = H // 2          # heads per core
KC = D_IN // 128     # k chunks
N_CORES = 8
PACK_P1 = False      # row-pack pass-1 head pairs via tile_position

_CACHE = {}


def build_bass():
    import concourse.bass as bass
    import concourse.bacc as bacc
    import concourse.mybir as mybir
    import concourse.tile as tile
    from contextlib import ExitStack

    f16 = mybir.dt.float16
    f32 = mybir.dt.float32
    AX = mybir.AxisListType
    ALU = mybir.AluOpType
    ACT_EXP = mybir.ActivationFunctionType.Exp

    nc = bacc.Bacc()
    xT_d = nc.declare_dram_parameter("xT", [D_IN, S], f16, isOutput=False)
    wq_d = nc.declare_dram_parameter("wq", [D_IN, HL * 64], f16, isOutput=False)
    wk_d = nc.declare_dram_parameter("wk", [D_IN, HL * 64], f16, isOutput=False)
    wv_d = nc.declare_dram_parameter("wv", [D_IN, HL * 64], f16, isOutput=False)
    zu_d = nc.declare_dram_parameter("zu", [HL, 65, S], f32, isOutput=True)

    with tile.TileContext(nc) as tc, ExitStack() as ctx:
        consts = ctx.enter_context(tc.tile_pool(name="consts", bufs=1))

        # ---- persistent SBUF tensors ----
        xT_sb = consts.tile([128, KC, S], f16)
        wq_sb = consts.tile([128, KC, HL * 64], f16)
        wk_sb = consts.tile([128, KC, HL * 64], f16)
        wv_sb = consts.tile([128, KC, HL * 64], f16)
        for c in range(KC):
            nc.sync.dma_start(out=xT_sb[:, c, :], in_=xT_d[c * 128:(c + 1) * 128, :])
            nc.sync.dma_start(out=wq_sb[:, c, :], in_=wq_d[c * 128:(c + 1) * 128, :])
            nc.sync.dma_start(out=wk_sb[:, c, :], in_=wk_d[c * 128:(c + 1) * 128, :])
            nc.sync.dma_start(out=wv_sb[:, c, :], in_=wv_d[c * 128:(c + 1) * 128, :])

        # per-head Q^T [64, S];  K~^T [65, S] with ones row;  V~ [128, 16, h, 65] with ones col
        qT = [consts.tile([64, S], f16, name=f"qT{h}", tag=f"qT{h}") for h in range(HL)]
        kT = [consts.tile([65, S], f16, name=f"kT{h}", tag=f"kT{h}") for h in range(HL)]
        v_all = consts.tile([128, 16, HL, 65], f16)
        for h in range(HL):
            nc.gpsimd.memset(kT[h][64:65, :], 1.0)
        nc.gpsimd.memset(v_all[:, :, :, 64:65], 1.0)

        # identity for the -max transpose (negation folded into the reduce)
        ident = consts.tile([128, 128], f32)
        nc.gpsimd.memset(ident, 0.0)
        nc.gpsimd.affine_select(
            out=ident, in_=ident,
            compare_op=ALU.not_equal, fill=1.0,
            base=0, pattern=[[-1, 128]], channel_multiplier=1,
        )

        # ---- phase A: QKV projections ----
        with tc.tile_pool(name="psA", bufs=2, space="PSUM") as psA:
            for pack in range(HL // 2):
                h0, h1 = 2 * pack, 2 * pack + 1
                for sc in range(S // 512):
                    ssl = slice(sc * 512, (sc + 1) * 512)
                    pq = psA.tile([128, 512], f32, tag="pq")
                    pk = psA.tile([128, 512], f32, tag="pk")
                    for c in range(KC):
                        nc.tensor.matmul(
                            pq, wq_sb[:, c, pack * 128:(pack + 1) * 128],
                            xT_sb[:, c, ssl], start=(c == 0), stop=(c == KC - 1))
                    for c in range(KC):
                        nc.tensor.matmul(
                            pk, wk_sb[:, c, pack * 128:(pack + 1) * 128],
                            xT_sb[:, c, ssl], start=(c == 0), stop=(c == KC - 1))
                    nc.scalar.copy(qT[h0][:, ssl], pq[0:64, :])
                    nc.scalar.copy(qT[h1][:, ssl], pq[64:128, :])
                    nc.vector.tensor_copy(kT[h0][0:64, ssl], pk[0:64, :])
                    nc.vector.tensor_copy(kT[h1][0:64, ssl], pk[64:128, :])
            for n_ in range(16):
                pv = psA.tile([128, HL * 64], f32, tag="pv")
                for c in range(KC):
                    nc.tensor.matmul(
                        pv, xT_sb[:, c, n_ * 128:(n_ + 1) * 128],
                        wv_sb[:, c, :], start=(c == 0), stop=(c == KC - 1))
                nc.vector.tensor_copy(
                    v_all[:, n_, :, 0:64],
                    pv.rearrange("p (h d) -> p h d", h=HL))

        # ---- phase B: attention ----
        with tc.tile_pool(name="p1", bufs=3, space="PSUM") as p1p, \
             tc.tile_pool(name="p2", bufs=2, space="PSUM") as p2p, \
             tc.tile_pool(name="ztp", bufs=1, space="PSUM") as ztp, \
             tc.tile_pool(name="sbp", bufs=3) as sbp, \
             tc.tile_pool(name="smp", bufs=2) as smp:
            for h in range(HL):
                for lc in range(S // 512):
                    lsl = slice(lc * 512, (lc + 1) * 512)
                    mcat = smp.tile([128, 4], f32, tag="mcat")
                    # pass-1: scores [l, n], row max per 128-l chunk
                    for ls in range(4):
                        l0 = lc * 512 + ls * 128
                        red = smp.tile([128, 4], f32, tag="red")
                        for j in range(4):
                            t1 = p1p.tile([128, 512], f32, tag="t1")
                            nc.tensor.matmul(
                                t1, qT[h][:, l0:l0 + 128],
                                kT[h][0:64, j * 512:(j + 1) * 512],
                                start=True, stop=True)
                            nc.vector.tensor_reduce(
                                red[:, j:j + 1], t1, axis=AX.X, op=ALU.max)
                        nc.vector.tensor_reduce(
                            mcat[:, ls:ls + 1], red, axis=AX.X, op=ALU.max,
                            negate=True)
                    # -max row -> fp16 -> row 64 of the assembled rhs
                    mneg = p2p.tile([4, 128], f32, tag="p2")
                    nc.tensor.transpose(mneg, mcat, ident)
                    negm = smp.tile([4, 128], f16, tag="negm")
                    nc.vector.tensor_copy(negm, mneg)
                    qasm = sbp.tile([65, 512], f16, tag="qasm")
                    nc.vector.tensor_copy(qasm[0:64, :], qT[h][:, lsl])
                    nc.sync.dma_start(out=qasm[64:65, :], in_=negm[:, :])
                    # pass-2 [n, l] with -max folded in, exp, attn@V
                    zt = ztp.tile([65, 512], f32, tag="zt")
                    for grp in range(8):
                        t2 = p2p.tile([128, 1024], f32, tag="p2")
                        for j in range(2):
                            n_ = grp * 2 + j
                            nc.tensor.matmul(
                                t2[:, j * 512:(j + 1) * 512],
                                kT[h][:, n_ * 128:(n_ + 1) * 128],
                                qasm, start=True, stop=True)
                        pt = sbp.tile([128, 1024], f16, tag="pt")
                        nc.scalar.activation(pt, t2, ACT_EXP)
                        for j in range(2):
                            n_ = grp * 2 + j
                            nc.tensor.matmul(
                                zt, v_all[:, n_, h, :],
                                pt[:, j * 512:(j + 1) * 512],
                                start=(n_ == 0), stop=(n_ == 15))
                    zsb = sbp.tile([65, 512], f32, tag="zsb")
                    nc.scalar.copy(zsb, zt)
                    nc.sync.dma_start(out=zu_d[h, :, lsl], in_=zsb)
    nc.finalize()
    return nc


def _get_nc():
    if "nc" not in _CACHE:
        _CACHE["nc"] = build_bass()
    return _CACHE["nc"]


def _prep_in_maps(x, wq, wk, wv):
    in_maps = []
    for c in range(N_CORES):
        b, half = c // 2, c % 2
        hs = range(half * HL, (half + 1) * HL)
        in_maps.append({
            "xT": np.ascontiguousarray(x[b].T).astype(np.float16),
            "wq": np.concatenate([wq[h] for h in hs], axis=1).astype(np.float32)
                    .__mul__(0.125).astype(np.float16),
            "wk": np.concatenate([wk[h] for h in hs], axis=1).astype(np.float16),
            "wv": np.concatenate([wv[h] for h in hs], axis=1).astype(np.float16),
        })
    return in_maps


def _postprocess(results, wc):
    out = np.empty((B, S, 64), np.float32)
    wcT = np.ascontiguousarray(wc.T).astype(np.float32)
    for b in range(B):
        zparts = []
        for half in range(2):
            zu = results[b * 2 + half]["zu"]          # [HL, 65, S] f32
            z = zu[:, :64, :] / zu[:, 64:65, :]       # [HL, 64, S]
            zparts.append(z.transpose(2, 0, 1).reshape(S, HL * 64))
        out[b] = np.concatenate(zparts, axis=1) @ wcT
    return out


def kernel(x, wq, wk, wv, wc):
    from concourse.bass_utils import run_bass_kernel_spmd
    nc = _get_nc()
    in_maps = _prep_in_maps(np.asarray(x), np.asarray(wq), np.asarray(wk), np.asarray(wv))
    res = run_bass_kernel_spmd(nc, in_maps, list(range(N_CORES))).results
    return _postprocess(res, np.asarray(wc))



# revision 22
# speedup vs baseline: 1.1184x; 1.1184x over previous
"""Multi-head attention TRN2 kernel (8 NeuronCores).

Problem: B=4, S=2048, D_IN=768, H=12, D_HEAD=64.
  q/k/v = einsum('hkd,bsk->bhsd', w{q,k,v}, x)
  out   = einsum('ij,bsj->bsi', wc, softmax(q k^T / 8) v  concat-heads)

Sharding: 8 cores = (batch b in 0..3) x (head-half in 0..1), 6 heads per core.
Each core computes, per head: Q^T,K^T [64,2048] fp16 projections, a
max-finding scores pass in [l,n] layout (per-block DVE max-reduces
straight from PSUM, with a few blocks per step offloaded to the scalar
engine as PSUM->fp16 copies + cheap 4x fp16 reduces), a second scores
pass in [n,l] layout with
the per-row max folded in via an augmented contraction row, exp on the
scalar engine (PSUM fp32 -> SBUF fp16), and attn@V with an appended
ones-column producing unnormalized z^T plus the softmax normalizer Z.
The host divides by Z, concatenates heads and applies the (tiny) output
projection in fp32 BLAS.

All matmuls run in fp16 (fp32 PSUM accumulation).
"""

import numpy as np

B, S, D_IN, H, D_HEAD = 4, 2048, 768, 12, 64
HL = H // 2          # heads per core
KC = D_IN // 128     # k chunks
N_CORES = 8

_CACHE = {}
CFG = {'shared_pool': False, 'zsb_engine': 'act', 'qasm_engine': 'dma', 'p1_prio': 30, 'act_blocks': frozenset({2, 10}), 'combine_engine': 'dve'}


def build_bass():
    import concourse.bass as bass
    import concourse.bacc as bacc
    import concourse.mybir as mybir
    import concourse.tile as tile
    from contextlib import ExitStack

    f16 = mybir.dt.float16
    f32 = mybir.dt.float32
    AX = mybir.AxisListType
    ALU = mybir.AluOpType
    ACT_EXP = mybir.ActivationFunctionType.Exp

    nc = bacc.Bacc()
    xT_d = nc.declare_dram_parameter("xT", [D_IN, S], f16, isOutput=False)
    wq_d = nc.declare_dram_parameter("wq", [D_IN, HL * 64], f16, isOutput=False)
    wk_d = nc.declare_dram_parameter("wk", [D_IN, HL * 64], f16, isOutput=False)
    wv_d = nc.declare_dram_parameter("wv", [D_IN, HL * 64], f16, isOutput=False)
    zu_d = nc.declare_dram_parameter("zu", [HL, 65, S], f32, isOutput=True)

    with tile.TileContext(nc) as tc, ExitStack() as ctx:
        consts = ctx.enter_context(tc.tile_pool(name="consts", bufs=1))

        # ---- persistent SBUF tensors ----
        xT_sb = consts.tile([128, KC, S], f16)
        wq_sb = consts.tile([128, KC, HL * 64], f16)
        wk_sb = consts.tile([128, KC, HL * 64], f16)
        wv_sb = consts.tile([128, KC, HL * 64], f16)
        for c in range(KC):
            nc.sync.dma_start(out=xT_sb[:, c, :], in_=xT_d[c * 128:(c + 1) * 128, :])
            nc.sync.dma_start(out=wq_sb[:, c, :], in_=wq_d[c * 128:(c + 1) * 128, :])
            nc.sync.dma_start(out=wk_sb[:, c, :], in_=wk_d[c * 128:(c + 1) * 128, :])
            nc.sync.dma_start(out=wv_sb[:, c, :], in_=wv_d[c * 128:(c + 1) * 128, :])

        # per-head Q^T [64, S];  K~^T [65, S] with ones row;  V~ [128, 16, h, 65] with ones col
        qT = [consts.tile([64, S], f16, name=f"qT{h}", tag=f"qT{h}") for h in range(HL)]
        kT = [consts.tile([65, S], f16, name=f"kT{h}", tag=f"kT{h}") for h in range(HL)]
        v_all = consts.tile([128, 16, HL, 65], f16)
        for h in range(HL):
            nc.gpsimd.memset(kT[h][64:65, :], 1.0)
        nc.gpsimd.memset(v_all[:, :, :, 64:65], 1.0)


        # ---- phase A: QKV projections ----
        with tc.tile_pool(name="psA", bufs=2, space="PSUM") as psA:
            for pack in range(HL // 2):
                h0, h1 = 2 * pack, 2 * pack + 1
                for sc in range(S // 512):
                    ssl = slice(sc * 512, (sc + 1) * 512)
                    pq = psA.tile([128, 512], f32, tag="pq")
                    pk = psA.tile([128, 512], f32, tag="pk")
                    for c in range(KC):
                        nc.tensor.matmul(
                            pq, wq_sb[:, c, pack * 128:(pack + 1) * 128],
                            xT_sb[:, c, ssl], start=(c == 0), stop=(c == KC - 1))
                    for c in range(KC):
                        nc.tensor.matmul(
                            pk, wk_sb[:, c, pack * 128:(pack + 1) * 128],
                            xT_sb[:, c, ssl], start=(c == 0), stop=(c == KC - 1))
                    nc.scalar.copy(qT[h0][:, ssl], pq[0:64, :])
                    nc.scalar.copy(qT[h1][:, ssl], pq[64:128, :])
                    nc.vector.tensor_copy(kT[h0][0:64, ssl], pk[0:64, :])
                    nc.vector.tensor_copy(kT[h1][0:64, ssl], pk[64:128, :])
            for n_ in range(16):
                pv = psA.tile([128, HL * 64], f32, tag="pv")
                for c in range(KC):
                    nc.tensor.matmul(
                        pv, xT_sb[:, c, n_ * 128:(n_ + 1) * 128],
                        wv_sb[:, c, :], start=(c == 0), stop=(c == KC - 1))
                nc.vector.tensor_copy(
                    v_all[:, n_, :, 0:64],
                    pv.rearrange("p (h d) -> p h d", h=HL))

        # ---- phase B: attention, software-pipelined ----
        # steps = (head, l-chunk).  For each step, pass-1 (max-finding) of
        # step i is interleaved at per-group granularity with pass-2
        # (exp + attn@V) of step i-1, so PE stalls in either stream are
        # filled by the other and ACT/DVE run concurrently.
        from contextlib import ExitStack as _ES
        _pools = _ES()
        if CFG['shared_pool']:
            psp = _pools.enter_context(tc.tile_pool(name="ps", bufs=3, space="PSUM"))
            p1p = p2p = psp
            ztp = _pools.enter_context(tc.tile_pool(name="ztp", bufs=2, space="PSUM"))
        else:
            p1p = _pools.enter_context(tc.tile_pool(name="p1", bufs=3, space="PSUM"))
            p2p = _pools.enter_context(tc.tile_pool(name="p2", bufs=2, space="PSUM"))
            ztp = _pools.enter_context(tc.tile_pool(name="ztp", bufs=1, space="PSUM"))
        with _pools, tc.tile_pool(name="sbp", bufs=3) as sbp, \
             tc.tile_pool(name="smp", bufs=2) as smp:

            def p1_chunk(h, lc, g, mcat, red):
                # pass-1 pair g (ls = g//2, n-half = g%2): two 512-n score
                # blocks, each max-reduced into a red column.  DVE can read
                # only one PSUM operand per instruction, so blocks reduce
                # independently; a few per step go via ACT copy + 4x fp16
                # reduce to balance engine load.  After the last block of an
                # ls, the four red columns collapse (negated) into mcat[:,ls].
                ls, half = divmod(g, 2)
                l0 = lc * 512 + ls * 128
                if CFG['shared_pool']:
                    # one 2-bank tile per chunk; a single wide reduce covers
                    # both 512-n blocks (same l rows), halving per-op PSUM
                    # access overhead
                    t1 = p1p.tile([128, 1024], f32, tag="ps", name="t1")
                    for j in range(2):
                        nc.tensor.matmul(
                            t1[:, j * 512:(j + 1) * 512], qT[h][:, l0:l0 + 128],
                            kT[h][0:64, (2 * half + j) * 512:(2 * half + j + 1) * 512],
                            start=True, stop=True)
                    nc.vector.tensor_reduce(
                        red[:, 2 * ls + half: 2 * ls + half + 1],
                        t1, axis=AX.X, op=ALU.max)
                    if half == 1:
                        nc.vector.tensor_reduce(
                            mcat[:, ls:ls + 1], red[:, 2 * ls:2 * ls + 2],
                            axis=AX.X, op=ALU.max, negate=True)
                else:
                    for j in range(2):
                        blk = g * 2 + j
                        tj = p1p.tile([128, 512], f32, tag="t1", name="tj")
                        nc.tensor.matmul(
                            tj, qT[h][:, l0:l0 + 128],
                            kT[h][0:64, (2 * half + j) * 512:(2 * half + j + 1) * 512],
                            start=True, stop=True)
                        if blk in CFG['act_blocks']:
                            cp = sbp.tile([128, 512], f16, tag="cp", name="cp")
                            nc.scalar.copy(cp, tj)
                            nc.vector.tensor_reduce(
                                red[:, 4 * ls + 2 * half + j: 4 * ls + 2 * half + j + 1],
                                cp, axis=AX.X, op=ALU.max)
                        else:
                            nc.vector.tensor_reduce(
                                red[:, 4 * ls + 2 * half + j: 4 * ls + 2 * half + j + 1],
                                tj, axis=AX.X, op=ALU.max)
                    if half == 1:
                        nc.vector.tensor_reduce(
                            mcat[:, ls:ls + 1], red[:, 4 * ls:4 * ls + 4],
                            axis=AX.X, op=ALU.max, negate=True)

            def p1_finish(h, lc, mcat, red):
                # -max cols [128, 0:4] -> XBAR DMA transpose -> mT[0:4, :]
                # holds -max for l = ls*128 + pidx -> row 64 of the rhs.
                # The qT rows are inserted by DMA to keep DVE/ACT off the
                # step-boundary critical path.
                mT = smp.tile([128, 128], f16, tag="mT")
                nc.sync.dma_start_transpose(mT, mcat)
                qasm = sbp.tile([65, 512], f16, tag="qasm")
                if CFG['qasm_engine'] == 'dma':
                    nc.sync.dma_start(out=qasm[0:64, :], in_=qT[h][:, lc * 512:(lc + 1) * 512])
                elif CFG['qasm_engine'] == 'dve':
                    nc.vector.tensor_copy(qasm[0:64, :], qT[h][:, lc * 512:(lc + 1) * 512])
                else:
                    nc.scalar.copy(qasm[0:64, :], qT[h][:, lc * 512:(lc + 1) * 512])
                nc.sync.dma_start(out=qasm[64:65, :], in_=mT[0:4, :])
                return qasm

            def p2_mm(h, g, qasm, pts):
                # pass-2 [n, l] with -max folded in + exp for n-pair g
                t2 = p2p.tile([128, 1024], f32, tag=("ps" if CFG['shared_pool'] else "p2"), name="t2")
                for j in range(2):
                    n_ = g * 2 + j
                    nc.tensor.matmul(
                        t2[:, j * 512:(j + 1) * 512],
                        kT[h][:, n_ * 128:(n_ + 1) * 128],
                        qasm, start=True, stop=True)
                pt = sbp.tile([128, 1024], f16, tag="pt")
                nc.scalar.activation(pt, t2, ACT_EXP)
                pts[g] = pt

            def p2_av(h, g, pts, zt):
                # attn@V for n-pair g (one group behind exp to hide latency)
                for j in range(2):
                    n_ = g * 2 + j
                    nc.tensor.matmul(
                        zt, v_all[:, n_, h, :],
                        pts[g][:, j * 512:(j + 1) * 512],
                        start=(n_ == 0), stop=(n_ == 15))

            def p2_finish(h, lc, zt):
                zsb = sbp.tile([65, 512], f32, tag="zsb")
                if CFG['zsb_engine'] == 'act':
                    nc.scalar.copy(zsb, zt)
                else:
                    nc.vector.tensor_copy(zsb, zt)
                nc.sync.dma_start(out=zu_d[h, :, lc * 512:(lc + 1) * 512], in_=zsb)

            steps = [(h, lc) for h in range(HL) for lc in range(S // 512)]
            prev = None          # (h, lc, qasm)
            for h, lc in steps:
                mcat = smp.tile([128, 128], f16, tag="mcat")
                red = smp.tile([128, 16], f16, tag="red")
                zt = ztp.tile([65, 512], f32, name="zt", tag="zt") if prev else None
                pts = {}
                for g in range(9):
                    if g < 8:
                        if CFG.get('p1_prio'):
                            with tc.high_priority(offset=CFG['p1_prio']):
                                p1_chunk(h, lc, g, mcat, red)
                        else:
                            p1_chunk(h, lc, g, mcat, red)
                        if prev is not None:
                            p2_mm(prev[0], g, prev[2], pts)
                    if prev is not None and g > 0:
                        p2_av(prev[0], g - 1, pts, zt)
                qasm = p1_finish(h, lc, mcat, red)
                if prev is not None:
                    p2_finish(prev[0], prev[1], zt)
                prev = (h, lc, qasm)
            # epilogue: pass-2 of the final step
            zt = ztp.tile([65, 512], f32, name="zt", tag="zt")
            pts = {}
            for g in range(9):
                if g < 8:
                    p2_mm(prev[0], g, prev[2], pts)
                if g > 0:
                    p2_av(prev[0], g - 1, pts, zt)
            p2_finish(prev[0], prev[1], zt)
    nc.finalize()
    return nc


def _get_nc():
    if "nc" not in _CACHE:
        _CACHE["nc"] = build_bass()
    return _CACHE["nc"]


def _prep_in_maps(x, wq, wk, wv):
    in_maps = []
    for c in range(N_CORES):
        b, half = c // 2, c % 2
        hs = range(half * HL, (half + 1) * HL)
        in_maps.append({
            "xT": np.ascontiguousarray(x[b].T).astype(np.float16),
            "wq": np.concatenate([wq[h] for h in hs], axis=1).astype(np.float32)
                    .__mul__(0.125).astype(np.float16),
            "wk": np.concatenate([wk[h] for h in hs], axis=1).astype(np.float16),
            "wv": np.concatenate([wv[h] for h in hs], axis=1).astype(np.float16),
        })
    return in_maps


def _postprocess(results, wc):
    out = np.empty((B, S, 64), np.float32)
    wcT = np.ascontiguousarray(wc.T).astype(np.float32)
    for b in range(B):
        zparts = []
        for half in range(2):
            zu = results[b * 2 + half]["zu"]          # [HL, 65, S] f32
            z = zu[:, :64, :] / zu[:, 64:65, :]       # [HL, 64, S]
            zparts.append(z.transpose(2, 0, 1).reshape(S, HL * 64))
        out[b] = np.concatenate(zparts, axis=1) @ wcT
    return out


def kernel(x, wq, wk, wv, wc):
    from concourse.bass_utils import run_bass_kernel_spmd
    nc = _get_nc()
    in_maps = _prep_in_maps(np.asarray(x), np.asarray(wq), np.asarray(wk), np.asarray(wv))
    res = run_bass_kernel_spmd(nc, in_maps, list(range(N_CORES))).results
    return _postprocess(res, np.asarray(wc))


# revision 34
# speedup vs baseline: 1.1637x; 1.0405x over previous
"""Multi-head attention TRN2 kernel (8 NeuronCores).

Problem: B=4, S=2048, D_IN=768, H=12, D_HEAD=64.
  q/k/v = einsum('hkd,bsk->bhsd', w{q,k,v}, x)
  out   = einsum('ij,bsj->bsi', wc, softmax(q k^T / 8) v  concat-heads)

Sharding: 8 cores = (batch b in 0..3) x (head-half in 0..1), 6 heads per core.
Each core computes, per head: Q^T,K^T [64,2048] fp16 projections, a
max-finding scores pass in [l,n] layout (per-block DVE max-reduces
straight from PSUM, with a few blocks per step offloaded to the scalar
engine as PSUM->fp16 copies + cheap 4x fp16 reduces), a second scores
pass in [n,l] layout with
the per-row max folded in via an augmented contraction row, exp on the
scalar engine (PSUM fp32 -> SBUF fp16), and attn@V with an appended
ones-column producing unnormalized z^T plus the softmax normalizer Z.
The host divides by Z, concatenates heads and applies the (tiny) output
projection in fp32 BLAS.

All matmuls run in fp16 (fp32 PSUM accumulation).
"""

import numpy as np

B, S, D_IN, H, D_HEAD = 4, 2048, 768, 12, 64
HL = H // 2          # heads per core
KC = D_IN // 128     # k chunks
N_CORES = 8

_CACHE = {}
CFG = {'shared_pool': False, 'zsb_engine': 'act', 'qasm_engine': 'dma', 'p1_prio': 30, 'act_blocks': frozenset(), 'combine_engine': 'dve', 'depth': 3, 'pre_skip': 0, 'v_first': True}


def build_bass():
    import concourse.bass as bass
    import concourse.bacc as bacc
    import concourse.mybir as mybir
    import concourse.tile as tile
    from contextlib import ExitStack

    f16 = mybir.dt.float16
    f32 = mybir.dt.float32
    AX = mybir.AxisListType
    ALU = mybir.AluOpType
    ACT_EXP = mybir.ActivationFunctionType.Exp

    nc = bacc.Bacc()
    xT_d = nc.declare_dram_parameter("xT", [D_IN, S], f16, isOutput=False)
    wq_d = nc.declare_dram_parameter("wq", [D_IN, HL * 64], f16, isOutput=False)
    wk_d = nc.declare_dram_parameter("wk", [D_IN, HL * 64], f16, isOutput=False)
    wv_d = nc.declare_dram_parameter("wv", [D_IN, HL * 64], f16, isOutput=False)
    zu_d = nc.declare_dram_parameter("zu", [HL, 65, S], f32, isOutput=True)

    with tile.TileContext(nc) as tc, ExitStack() as ctx:
        consts = ctx.enter_context(tc.tile_pool(name="consts", bufs=1))

        # ---- persistent SBUF tensors ----
        xT_sb = consts.tile([128, KC, S], f16)
        wq_sb = consts.tile([128, KC, HL * 64], f16)
        wk_sb = consts.tile([128, KC, HL * 64], f16)
        wv_sb = consts.tile([128, KC, HL * 64], f16)
        for c in range(KC):
            nc.sync.dma_start(out=xT_sb[:, c, :], in_=xT_d[c * 128:(c + 1) * 128, :])
            nc.sync.dma_start(out=wq_sb[:, c, :], in_=wq_d[c * 128:(c + 1) * 128, :])
            nc.sync.dma_start(out=wk_sb[:, c, :], in_=wk_d[c * 128:(c + 1) * 128, :])
        for c in range(KC):
            nc.sync.dma_start(out=wv_sb[:, c, :], in_=wv_d[c * 128:(c + 1) * 128, :])

        # per-head Q^T [64, S];  K~^T [65, S] with ones row;  V~ [128, 16, h, 65] with ones col
        qT = [consts.tile([64, S], f16, name=f"qT{h}", tag=f"qT{h}") for h in range(HL)]
        kT = [consts.tile([65, S], f16, name=f"kT{h}", tag=f"kT{h}") for h in range(HL)]
        v_all = consts.tile([128, 16, HL, 65], f16)
        for h in range(HL):
            nc.gpsimd.memset(kT[h][64:65, :], 1.0)
        nc.gpsimd.memset(v_all[:, :, :, 64:65], 1.0)


        # ---- attention, software-pipelined at depth D over phase A ----
        # steps = (head, l-chunk).  pass-1 (max-finding, DVE-bound) of step
        # i+1 is interleaved at per-group granularity with pass-2
        # (exp + attn@V) of step i.  Additionally the first D steps' pass-1
        # is emitted under the phase-A projection matmuls (PE-bound): pq/pk
        # are single-buffered so psA takes only 4 banks, and the 3-bank
        # pass-1 pool opens alongside it.  After psA closes, pass-2's 4
        # banks + zt open in the freed space.
        D = CFG['depth']
        from contextlib import ExitStack as _ES
        _pools = _ES()
        p1p = _pools.enter_context(tc.tile_pool(name="p1", bufs=3, space="PSUM"))
        with _pools, tc.tile_pool(name="sbp", bufs=3) as sbp, \
             tc.tile_pool(name="qsp", bufs=D + 2) as qsp, \
             tc.tile_pool(name="smp", bufs=D + 1) as smp:

            def p1_chunk(h, lc, g, mcat, red):
                # pass-1 pair g (ls = g//2, n-half = g%2): two 512-n score
                # blocks, each max-reduced into a red column.  DVE can read
                # only one PSUM operand per instruction, so blocks reduce
                # independently; a few per step go via ACT copy + fp16
                # reduce to balance engine load.  After the last block of an
                # ls, the four red columns collapse (negated) into mcat[:,ls].
                ls, half = divmod(g, 2)
                l0 = lc * 512 + ls * 128
                for j in range(2):
                    blk = g * 2 + j
                    tj = p1p.tile([128, 512], f32, tag="t1", name="tj")
                    nc.tensor.matmul(
                        tj, qT[h][:, l0:l0 + 128],
                        kT[h][0:64, (2 * half + j) * 512:(2 * half + j + 1) * 512],
                        start=True, stop=True)
                    if blk in CFG['act_blocks']:
                        cp = sbp.tile([128, 512], f16, tag="cp", name="cp")
                        nc.scalar.copy(cp, tj)
                        nc.vector.tensor_reduce(
                            red[:, 4 * ls + 2 * half + j: 4 * ls + 2 * half + j + 1],
                            cp, axis=AX.X, op=ALU.max)
                    else:
                        nc.vector.tensor_reduce(
                            red[:, 4 * ls + 2 * half + j: 4 * ls + 2 * half + j + 1],
                            tj, axis=AX.X, op=ALU.max)
                if half == 1:
                    nc.vector.tensor_reduce(
                        mcat[:, ls:ls + 1], red[:, 4 * ls:4 * ls + 4],
                        axis=AX.X, op=ALU.max, negate=True)

            def p1_finish(h, lc, mcat, red):
                # -max cols [128, 0:4] -> XBAR DMA transpose -> mT[0:4, :]
                # holds -max for l = ls*128 + pidx -> row 64 of the rhs.
                # The qT rows are inserted by DMA to keep DVE/ACT off the
                # step-boundary critical path.
                mT = smp.tile([128, 128], f16, tag="mT")
                nc.sync.dma_start_transpose(mT, mcat)
                qasm = qsp.tile([65, 512], f16, tag="qasm")
                if CFG['qasm_engine'] == 'dma':
                    nc.sync.dma_start(out=qasm[0:64, :], in_=qT[h][:, lc * 512:(lc + 1) * 512])
                elif CFG['qasm_engine'] == 'dve':
                    nc.vector.tensor_copy(qasm[0:64, :], qT[h][:, lc * 512:(lc + 1) * 512])
                else:
                    nc.scalar.copy(qasm[0:64, :], qT[h][:, lc * 512:(lc + 1) * 512])
                nc.sync.dma_start(out=qasm[64:65, :], in_=mT[0:4, :])
                return qasm

            pools = {}

            def p2_mm(h, g, qasm, pts):
                # pass-2 [n, l] with -max folded in + exp for n-pair g
                t2 = pools['p2'].tile([128, 1024], f32, tag="p2", name="t2")
                for j in range(2):
                    n_ = g * 2 + j
                    nc.tensor.matmul(
                        t2[:, j * 512:(j + 1) * 512],
                        kT[h][:, n_ * 128:(n_ + 1) * 128],
                        qasm, start=True, stop=True)
                pt = sbp.tile([128, 1024], f16, tag="pt")
                nc.scalar.activation(pt, t2, ACT_EXP)
                pts[g] = pt

            def p2_av(h, g, pts, zt):
                # attn@V for n-pair g (one group behind exp to hide latency)
                for j in range(2):
                    n_ = g * 2 + j
                    nc.tensor.matmul(
                        zt, v_all[:, n_, h, :],
                        pts[g][:, j * 512:(j + 1) * 512],
                        start=(n_ == 0), stop=(n_ == 15))

            def p2_finish(h, lc, zt, last=False):
                zsb = sbp.tile([65, 512], f32, tag="zsb")
                if CFG['zsb_engine'] == 'act' and not last:
                    nc.scalar.copy(zsb, zt)
                else:
                    nc.vector.tensor_copy(zsb, zt)
                nc.sync.dma_start(out=zu_d[h, :, lc * 512:(lc + 1) * 512], in_=zsb)

            steps = [(h, lc) for h in range(HL) for lc in range(S // 512)]
            NS = len(steps)
            qasms = {}           # step index -> qasm tile
            p1_state = {}        # step index -> (mcat, red)
            p1_done = 0          # count of fully-emitted pass-1 steps

            def emit_p1_step_chunk(i, g):
                # emit pass-1 chunk g of step i (allocating state at g==0),
                # finishing with the transpose/qasm assembly after g==7
                h, lc = steps[i]
                if g == 0:
                    p1_state[i] = (smp.tile([128, 128], f16, name="mcat", tag="mcat"),
                                   smp.tile([128, 16], f16, name="red", tag="red"))
                mcat, red = p1_state[i]
                p1_chunk(h, lc, g, mcat, red)
                if g == 7:
                    qasms[i] = p1_finish(h, lc, mcat, red)
                    del p1_state[i]

            # ---- phase A: projections + pass-1 of the first D steps ----
            with tc.tile_pool(name="psA", bufs=1, space="PSUM") as psAq, \
                 tc.tile_pool(name="psAv", bufs=2, space="PSUM") as psAv:

                def qk_unit(pack, sc):
                    h0, h1 = 2 * pack, 2 * pack + 1
                    ssl = slice(sc * 512, (sc + 1) * 512)
                    pq = psAq.tile([128, 512], f32, tag="pq")
                    for c in range(KC):
                        nc.tensor.matmul(
                            pq, wq_sb[:, c, pack * 128:(pack + 1) * 128],
                            xT_sb[:, c, ssl], start=(c == 0), stop=(c == KC - 1))
                    pk = psAq.tile([128, 512], f32, tag="pk")
                    for c in range(KC):
                        nc.tensor.matmul(
                            pk, wk_sb[:, c, pack * 128:(pack + 1) * 128],
                            xT_sb[:, c, ssl], start=(c == 0), stop=(c == KC - 1))
                    nc.scalar.copy(qT[h0][:, ssl], pq[0:64, :])
                    nc.scalar.copy(qT[h1][:, ssl], pq[64:128, :])
                    nc.scalar.copy(kT[h0][0:64, ssl], pk[0:64, :])
                    nc.scalar.copy(kT[h1][0:64, ssl], pk[64:128, :])

                def v_unit(n_):
                    pv = psAv.tile([128, HL * 64], f32, tag="pv")
                    for c in range(KC):
                        nc.tensor.matmul(
                            pv, xT_sb[:, c, n_ * 128:(n_ + 1) * 128],
                            wv_sb[:, c, :], start=(c == 0), stop=(c == KC - 1))
                    nc.vector.tensor_copy(
                        v_all[:, n_, :, 0:64],
                        pv.rearrange("p (h d) -> p h d", h=HL))

                for sc in range(S // 512):
                    qk_unit(0, sc)
                if CFG.get('v_first'):
                    units = ([("v", n_, None) for n_ in range(16)]
                             + [("qk", 1, sc) for sc in range(S // 512)]
                             + [("qk", 2, sc) for sc in range(S // 512)])
                else:
                    units = ([("qk", 1, sc) for sc in range(S // 512)]
                             + [("qk", 2, sc) for sc in range(S // 512)]
                             + [("v", n_, None) for n_ in range(16)])
                pre_chunks = [(i, g) for i in range(min(D, 8)) for g in range(8)]
                ci = 0
                skip = CFG.get('pre_skip', 6)
                for ui, u in enumerate(units):
                    if u[0] == "qk":
                        qk_unit(u[1], u[2])
                    else:
                        v_unit(u[1])
                    if ui < skip:
                        continue
                    want = (ui + 1 - skip) * len(pre_chunks) // max(1, len(units) - skip)
                    while ci < want:
                        i, g = pre_chunks[ci]
                        emit_p1_step_chunk(i, g)
                        ci += 1
                while ci < len(pre_chunks):
                    i, g = pre_chunks[ci]
                    emit_p1_step_chunk(i, g)
                    ci += 1
                p1_done = min(D, 8)

            # ---- phase B ----
            with tc.tile_pool(name="p2", bufs=2, space="PSUM") as p2p_, \
                 tc.tile_pool(name="ztp", bufs=1, space="PSUM") as ztp:
                pools['p2'] = p2p_
                for i in range(NS):
                    h, lc = steps[i]
                    zt = ztp.tile([65, 512], f32, name="zt", tag="zt")
                    pts = {}
                    nxt = i + D
                    for g in range(9):
                        if g < 8:
                            if nxt < NS:
                                if CFG.get('p1_prio'):
                                    with tc.high_priority(offset=CFG['p1_prio']):
                                        emit_p1_step_chunk(nxt, g)
                                else:
                                    emit_p1_step_chunk(nxt, g)
                            p2_mm(h, g, qasms[i], pts)
                        if g > 0:
                            p2_av(h, g - 1, pts, zt)
                    p2_finish(h, lc, zt, last=(i >= NS - 3))
                    del qasms[i]
    nc.finalize()
    return nc


def _get_nc():
    if "nc" not in _CACHE:
        _CACHE["nc"] = build_bass()
    return _CACHE["nc"]


def _prep_in_maps(x, wq, wk, wv):
    in_maps = []
    for c in range(N_CORES):
        b, half = c // 2, c % 2
        hs = range(half * HL, (half + 1) * HL)
        in_maps.append({
            "xT": np.ascontiguousarray(x[b].T).astype(np.float16),
            "wq": np.concatenate([wq[h] for h in hs], axis=1).astype(np.float32)
                    .__mul__(0.125).astype(np.float16),
            "wk": np.concatenate([wk[h] for h in hs], axis=1).astype(np.float16),
            "wv": np.concatenate([wv[h] for h in hs], axis=1).astype(np.float16),
        })
    return in_maps


def _postprocess(results, wc):
    out = np.empty((B, S, 64), np.float32)
    wcT = np.ascontiguousarray(wc.T).astype(np.float32)
    for b in range(B):
        zparts = []
        for half in range(2):
            zu = results[b * 2 + half]["zu"]          # [HL, 65, S] f32
            z = zu[:, :64, :] / zu[:, 64:65, :]       # [HL, 64, S]
            zparts.append(z.transpose(2, 0, 1).reshape(S, HL * 64))
        out[b] = np.concatenate(zparts, axis=1) @ wcT
    return out


def kernel(x, wq, wk, wv, wc):
    from concourse.bass_utils import run_bass_kernel_spmd
    nc = _get_nc()
    in_maps = _prep_in_maps(np.asarray(x), np.asarray(wq), np.asarray(wk), np.asarray(wv))
    res = run_bass_kernel_spmd(nc, in_maps, list(range(N_CORES))).results
    return _postprocess(res, np.asarray(wc))


# revision 36
# speedup vs baseline: 1.1660x; 1.0020x over previous
"""Multi-head attention TRN2 kernel (8 NeuronCores).

Problem: B=4, S=2048, D_IN=768, H=12, D_HEAD=64.
  q/k/v = einsum('hkd,bsk->bhsd', w{q,k,v}, x)
  out   = einsum('ij,bsj->bsi', wc, softmax(q k^T / 8) v  concat-heads)

Sharding: 8 cores = (batch b in 0..3) x (head-half in 0..1), 6 heads per core.
Each core computes, per head: Q^T,K^T [64,2048] fp16 projections, a
max-finding scores pass in [l,n] layout (per-block DVE max-reduces
straight from PSUM, with a few blocks per step offloaded to the scalar
engine as PSUM->fp16 copies + cheap 4x fp16 reduces), a second scores
pass in [n,l] layout with
the per-row max folded in via an augmented contraction row, exp on the
scalar engine (PSUM fp32 -> SBUF fp16), and attn@V with an appended
ones-column producing unnormalized z^T plus the softmax normalizer Z.
The host divides by Z, concatenates heads and applies the (tiny) output
projection in fp32 BLAS.

All matmuls run in fp16 (fp32 PSUM accumulation).
"""

import numpy as np

B, S, D_IN, H, D_HEAD = 4, 2048, 768, 12, 64
HL = H // 2          # heads per core
KC = D_IN // 128     # k chunks
N_CORES = 8

_CACHE = {}
CFG = {'shared_pool': False, 'zsb_engine': 'act', 'qasm_engine': 'dma', 'p1_prio': 30, 'act_blocks': frozenset(), 'combine_engine': 'dve', 'depth': 3, 'pre_skip': 0, 'v_first': True, 'sbp_bufs': 3, 'psav_bufs': 3, 'k_copy_dve': False}


def build_bass():
    import concourse.bass as bass
    import concourse.bacc as bacc
    import concourse.mybir as mybir
    import concourse.tile as tile
    from contextlib import ExitStack

    f16 = mybir.dt.float16
    f32 = mybir.dt.float32
    AX = mybir.AxisListType
    ALU = mybir.AluOpType
    ACT_EXP = mybir.ActivationFunctionType.Exp

    nc = bacc.Bacc()
    xT_d = nc.declare_dram_parameter("xT", [D_IN, S], f16, isOutput=False)
    wq_d = nc.declare_dram_parameter("wq", [D_IN, HL * 64], f16, isOutput=False)
    wk_d = nc.declare_dram_parameter("wk", [D_IN, HL * 64], f16, isOutput=False)
    wv_d = nc.declare_dram_parameter("wv", [D_IN, HL * 64], f16, isOutput=False)
    zu_d = nc.declare_dram_parameter("zu", [HL, 65, S], f32, isOutput=True)

    with tile.TileContext(nc) as tc, ExitStack() as ctx:
        consts = ctx.enter_context(tc.tile_pool(name="consts", bufs=1))

        # ---- persistent SBUF tensors ----
        xT_sb = consts.tile([128, KC, S], f16)
        wq_sb = consts.tile([128, KC, HL * 64], f16)
        wk_sb = consts.tile([128, KC, HL * 64], f16)
        wv_sb = consts.tile([128, KC, HL * 64], f16)
        for c in range(KC):
            nc.sync.dma_start(out=xT_sb[:, c, :], in_=xT_d[c * 128:(c + 1) * 128, :])
            nc.sync.dma_start(out=wq_sb[:, c, :], in_=wq_d[c * 128:(c + 1) * 128, :])
            nc.sync.dma_start(out=wk_sb[:, c, :], in_=wk_d[c * 128:(c + 1) * 128, :])
        for c in range(KC):
            nc.sync.dma_start(out=wv_sb[:, c, :], in_=wv_d[c * 128:(c + 1) * 128, :])

        # per-head Q^T [64, S];  K~^T [65, S] with ones row;  V~ [128, 16, h, 65] with ones col
        qT = [consts.tile([64, S], f16, name=f"qT{h}", tag=f"qT{h}") for h in range(HL)]
        kT = [consts.tile([65, S], f16, name=f"kT{h}", tag=f"kT{h}") for h in range(HL)]
        v_all = consts.tile([128, 16, HL, 65], f16)
        for h in range(HL):
            nc.gpsimd.memset(kT[h][64:65, :], 1.0)
        nc.gpsimd.memset(v_all[:, :, :, 64:65], 1.0)


        # ---- attention, software-pipelined at depth D over phase A ----
        # steps = (head, l-chunk).  pass-1 (max-finding, DVE-bound) of step
        # i+1 is interleaved at per-group granularity with pass-2
        # (exp + attn@V) of step i.  Additionally the first D steps' pass-1
        # is emitted under the phase-A projection matmuls (PE-bound): pq/pk
        # are single-buffered so psA takes only 4 banks, and the 3-bank
        # pass-1 pool opens alongside it.  After psA closes, pass-2's 4
        # banks + zt open in the freed space.
        D = CFG['depth']
        from contextlib import ExitStack as _ES
        _pools = _ES()
        p1p = _pools.enter_context(tc.tile_pool(name="p1", bufs=3, space="PSUM"))
        with _pools, tc.tile_pool(name="sbp", bufs=CFG.get("sbp_bufs", 3)) as sbp, \
             tc.tile_pool(name="qsp", bufs=D + 2) as qsp, \
             tc.tile_pool(name="smp", bufs=D + 1) as smp:

            def p1_chunk(h, lc, g, mcat, red):
                # pass-1 pair g (ls = g//2, n-half = g%2): two 512-n score
                # blocks, each max-reduced into a red column.  DVE can read
                # only one PSUM operand per instruction, so blocks reduce
                # independently; a few per step go via ACT copy + fp16
                # reduce to balance engine load.  After the last block of an
                # ls, the four red columns collapse (negated) into mcat[:,ls].
                ls, half = divmod(g, 2)
                l0 = lc * 512 + ls * 128
                for j in range(2):
                    blk = g * 2 + j
                    tj = p1p.tile([128, 512], f32, tag="t1", name="tj")
                    nc.tensor.matmul(
                        tj, qT[h][:, l0:l0 + 128],
                        kT[h][0:64, (2 * half + j) * 512:(2 * half + j + 1) * 512],
                        start=True, stop=True)
                    if blk in CFG['act_blocks']:
                        cp = sbp.tile([128, 512], f16, tag="cp", name="cp")
                        nc.scalar.copy(cp, tj)
                        nc.vector.tensor_reduce(
                            red[:, 4 * ls + 2 * half + j: 4 * ls + 2 * half + j + 1],
                            cp, axis=AX.X, op=ALU.max)
                    else:
                        nc.vector.tensor_reduce(
                            red[:, 4 * ls + 2 * half + j: 4 * ls + 2 * half + j + 1],
                            tj, axis=AX.X, op=ALU.max)
                if half == 1:
                    nc.vector.tensor_reduce(
                        mcat[:, ls:ls + 1], red[:, 4 * ls:4 * ls + 4],
                        axis=AX.X, op=ALU.max, negate=True)

            def p1_finish(h, lc, mcat, red):
                # -max cols [128, 0:4] -> XBAR DMA transpose -> mT[0:4, :]
                # holds -max for l = ls*128 + pidx -> row 64 of the rhs.
                # The qT rows are inserted by DMA to keep DVE/ACT off the
                # step-boundary critical path.
                mT = smp.tile([128, 128], f16, tag="mT")
                nc.sync.dma_start_transpose(mT, mcat)
                qasm = qsp.tile([65, 512], f16, tag="qasm")
                if CFG['qasm_engine'] == 'dma':
                    nc.sync.dma_start(out=qasm[0:64, :], in_=qT[h][:, lc * 512:(lc + 1) * 512])
                elif CFG['qasm_engine'] == 'dve':
                    nc.vector.tensor_copy(qasm[0:64, :], qT[h][:, lc * 512:(lc + 1) * 512])
                else:
                    nc.scalar.copy(qasm[0:64, :], qT[h][:, lc * 512:(lc + 1) * 512])
                nc.sync.dma_start(out=qasm[64:65, :], in_=mT[0:4, :])
                return qasm

            pools = {}

            def p2_mm(h, g, qasm, pts):
                # pass-2 [n, l] with -max folded in + exp for n-pair g
                t2 = pools['p2'].tile([128, 1024], f32, tag="p2", name="t2")
                for j in range(2):
                    n_ = g * 2 + j
                    nc.tensor.matmul(
                        t2[:, j * 512:(j + 1) * 512],
                        kT[h][:, n_ * 128:(n_ + 1) * 128],
                        qasm, start=True, stop=True)
                pt = sbp.tile([128, 1024], f16, tag="pt")
                nc.scalar.activation(pt, t2, ACT_EXP)
                pts[g] = pt

            def p2_av(h, g, pts, zt):
                # attn@V for n-pair g (one group behind exp to hide latency)
                for j in range(2):
                    n_ = g * 2 + j
                    nc.tensor.matmul(
                        zt, v_all[:, n_, h, :],
                        pts[g][:, j * 512:(j + 1) * 512],
                        start=(n_ == 0), stop=(n_ == 15))

            def p2_finish(h, lc, zt, last=False):
                zsb = sbp.tile([65, 512], f32, tag="zsb")
                if CFG['zsb_engine'] == 'act' and not last:
                    nc.scalar.copy(zsb, zt)
                else:
                    nc.vector.tensor_copy(zsb, zt)
                nc.sync.dma_start(out=zu_d[h, :, lc * 512:(lc + 1) * 512], in_=zsb)

            steps = [(h, lc) for h in range(HL) for lc in range(S // 512)]
            NS = len(steps)
            qasms = {}           # step index -> qasm tile
            p1_state = {}        # step index -> (mcat, red)
            p1_done = 0          # count of fully-emitted pass-1 steps

            def emit_p1_step_chunk(i, g):
                # emit pass-1 chunk g of step i (allocating state at g==0),
                # finishing with the transpose/qasm assembly after g==7
                h, lc = steps[i]
                if g == 0:
                    p1_state[i] = (smp.tile([128, 128], f16, name="mcat", tag="mcat"),
                                   smp.tile([128, 16], f16, name="red", tag="red"))
                mcat, red = p1_state[i]
                p1_chunk(h, lc, g, mcat, red)
                if g == 7:
                    qasms[i] = p1_finish(h, lc, mcat, red)
                    del p1_state[i]

            # ---- phase A: projections + pass-1 of the first D steps ----
            with tc.tile_pool(name="psA", bufs=1, space="PSUM") as psAq, \
                 tc.tile_pool(name="psAv", bufs=CFG.get("psav_bufs", 2), space="PSUM") as psAv:

                def qk_unit(pack, sc):
                    h0, h1 = 2 * pack, 2 * pack + 1
                    ssl = slice(sc * 512, (sc + 1) * 512)
                    pq = psAq.tile([128, 512], f32, tag="pq")
                    for c in range(KC):
                        nc.tensor.matmul(
                            pq, wq_sb[:, c, pack * 128:(pack + 1) * 128],
                            xT_sb[:, c, ssl], start=(c == 0), stop=(c == KC - 1))
                    pk = psAq.tile([128, 512], f32, tag="pk")
                    for c in range(KC):
                        nc.tensor.matmul(
                            pk, wk_sb[:, c, pack * 128:(pack + 1) * 128],
                            xT_sb[:, c, ssl], start=(c == 0), stop=(c == KC - 1))
                    nc.scalar.copy(qT[h0][:, ssl], pq[0:64, :])
                    nc.scalar.copy(qT[h1][:, ssl], pq[64:128, :])
                    if CFG.get('k_copy_dve'):
                        nc.vector.tensor_copy(kT[h0][0:64, ssl], pk[0:64, :])
                        nc.vector.tensor_copy(kT[h1][0:64, ssl], pk[64:128, :])
                    else:
                        nc.scalar.copy(kT[h0][0:64, ssl], pk[0:64, :])
                        nc.scalar.copy(kT[h1][0:64, ssl], pk[64:128, :])

                def v_unit(n_):
                    pv = psAv.tile([128, HL * 64], f32, tag="pv")
                    for c in range(KC):
                        nc.tensor.matmul(
                            pv, xT_sb[:, c, n_ * 128:(n_ + 1) * 128],
                            wv_sb[:, c, :], start=(c == 0), stop=(c == KC - 1))
                    nc.vector.tensor_copy(
                        v_all[:, n_, :, 0:64],
                        pv.rearrange("p (h d) -> p h d", h=HL))

                for sc in range(S // 512):
                    qk_unit(0, sc)
                if CFG.get('v_first'):
                    units = ([("v", n_, None) for n_ in range(16)]
                             + [("qk", 1, sc) for sc in range(S // 512)]
                             + [("qk", 2, sc) for sc in range(S // 512)])
                else:
                    units = ([("qk", 1, sc) for sc in range(S // 512)]
                             + [("qk", 2, sc) for sc in range(S // 512)]
                             + [("v", n_, None) for n_ in range(16)])
                pre_chunks = [(i, g) for i in range(min(D, 8)) for g in range(8)]
                ci = 0
                skip = CFG.get('pre_skip', 6)
                for ui, u in enumerate(units):
                    if u[0] == "qk":
                        qk_unit(u[1], u[2])
                    else:
                        v_unit(u[1])
                    if ui < skip:
                        continue
                    want = (ui + 1 - skip) * len(pre_chunks) // max(1, len(units) - skip)
                    while ci < want:
                        i, g = pre_chunks[ci]
                        emit_p1_step_chunk(i, g)
                        ci += 1
                while ci < len(pre_chunks):
                    i, g = pre_chunks[ci]
                    emit_p1_step_chunk(i, g)
                    ci += 1
                p1_done = min(D, 8)

            # ---- phase B ----
            with tc.tile_pool(name="p2", bufs=2, space="PSUM") as p2p_, \
                 tc.tile_pool(name="ztp", bufs=1, space="PSUM") as ztp:
                pools['p2'] = p2p_
                for i in range(NS):
                    h, lc = steps[i]
                    zt = ztp.tile([65, 512], f32, name="zt", tag="zt")
                    pts = {}
                    nxt = i + D
                    for g in range(9):
                        if g < 8:
                            if nxt < NS:
                                if CFG.get('p1_prio'):
                                    with tc.high_priority(offset=CFG['p1_prio']):
                                        emit_p1_step_chunk(nxt, g)
                                else:
                                    emit_p1_step_chunk(nxt, g)
                            p2_mm(h, g, qasms[i], pts)
                        if g > 0:
                            p2_av(h, g - 1, pts, zt)
                    p2_finish(h, lc, zt, last=(i >= NS - 3))
                    del qasms[i]
    nc.finalize()
    return nc


def _get_nc():
    if "nc" not in _CACHE:
        _CACHE["nc"] = build_bass()
    return _CACHE["nc"]


def _prep_in_maps(x, wq, wk, wv):
    in_maps = []
    for c in range(N_CORES):
        b, half = c // 2, c % 2
        hs = range(half * HL, (half + 1) * HL)
        in_maps.append({
            "xT": np.ascontiguousarray(x[b].T).astype(np.float16),
            "wq": np.concatenate([wq[h] for h in hs], axis=1).astype(np.float32)
                    .__mul__(0.125).astype(np.float16),
            "wk": np.concatenate([wk[h] for h in hs], axis=1).astype(np.float16),
            "wv": np.concatenate([wv[h] for h in hs], axis=1).astype(np.float16),
        })
    return in_maps


def _postprocess(results, wc):
    out = np.empty((B, S, 64), np.float32)
    wcT = np.ascontiguousarray(wc.T).astype(np.float32)
    for b in range(B):
        zparts = []
        for half in range(2):
            zu = results[b * 2 + half]["zu"]          # [HL, 65, S] f32
            z = zu[:, :64, :] / zu[:, 64:65, :]       # [HL, 64, S]
            zparts.append(z.transpose(2, 0, 1).reshape(S, HL * 64))
        out[b] = np.concatenate(zparts, axis=1) @ wcT
    return out


def kernel(x, wq, wk, wv, wc):
    from concourse.bass_utils import run_bass_kernel_spmd
    nc = _get_nc()
    in_maps = _prep_in_maps(np.asarray(x), np.asarray(wq), np.asarray(wk), np.asarray(wv))
    res = run_bass_kernel_spmd(nc, in_maps, list(range(N_CORES))).results
    return _postprocess(res, np.asarray(wc))
